# revision 2
# baseline (speedup 1.0000x reference)
"""CVTGAD loss kernel for 8 TRN2 NeuronCores.

Math (matches the jax reference):
  l_node[i,g] = ln(sum_j exp(simT[j,i])) - simT[i,i]   per graph (128x128 InfoNCE)
  l_graph     = InfoNCE over pooled graph embeddings (64 own rows x 512)
  out = (std(l_node)+1e-6) * mean(l_node) + (std(l_graph)+1e-6) * mean(l_graph)

Sharding: 64 graphs (8192 node rows) per core; h_s_final replicated (rolled per
core so each core's own graphs sit at columns 0:64, making the SPMD diag mask
core-independent). Device emits ln-rowsum and pos columns [128,64] plus
l_graph [64]; host does the tiny mean/std/weighted-sum epilogue.

Kernel strategy per core (tuned against the TimelineSim cost model):
  - SWDGE DMA-cast loads f32->bf16 (HBM reads billed on bf16 out bytes).
  - Row sumsq in ONE DVE pass per graph via scalar_tensor_tensor
    (out=(h*1)*h, accum_out=sum) -- InstTensorScalarPtr runs in 4x mode.
  - inv norms as exp(-0.5*ln(x)) on ACT, batched per block; 1/tau folded
    into hf's inv via the ln-bias trick.
  - hf normalized explicitly (DVE tensor_scalar 4x); hs stays raw and its
    1/|hs_j| rides the per-partition scale AP of the per-graph ACT Exp.
  - Gram computed TRANSPOSED (lhsT=hs^T chunk): s_ps[j,i] so that
  - rowsum over j = PE matmul with lhsT=exp_sb, rhs=ones -> psum column
    [128,1] per graph (ap_size=1: nearly free on the PE).
  - diagonal (pos) in ONE DVE 4x pass: scalar_tensor_tensor with in1=identity,
    accum_out = exp(pos).
  - l columns = ln(rowsum_psum) - ln(exppos) built with 2 batched ACT lns +
    1 DVE subtract; single [128,64] f32 store.
  - All activations pinned to the natural_log_exp_and_others table set
    (single ACT_TABLE_LOAD; the default picker thrashes ~2.7us reloads).
"""

import numpy as np

B = 512
NPER = 128
D = 256
NCORES = 8
GPC = B // NCORES      # 64 graphs per core
BLK = 8                # graphs per DMA block
NBLK = GPC // BLK
QG = 4                 # graphs per PSUM group (gram/exp granularity)
TAU = 0.5
LN_INV_TAU = float(np.log(1.0 / TAU))

_CACHE = {}


def _build():
    import os
    import ml_dtypes
    import concourse.bacc as bacc
    import concourse.tile as tile
    import concourse.mybir as mybir
    import concourse.hw_specs as hw_specs
    from concourse._compat import get_trn_type

    # Pin every activation to the one table set that has Exp+Ln+Square+Copy,
    # so the compiler emits a single ACT_TABLE_LOAD instead of thrashing
    # (each reload costs ~2.7us and the default picker alternates sets).
    if not getattr(hw_specs, "_nle_patched", False):
        _orig_tables = hw_specs.get_activation_tables

        def _only_nle(arch):
            t = _orig_tables(arch)
            keep = "natural_log_exp_and_others"
            return {k: (v if k == keep else set()) for k, v in t.items()}

        hw_specs.get_activation_tables = _only_nle
        bacc.get_activation_tables = _only_nle
        hw_specs._nle_patched = True

    f32 = mybir.dt.float32
    bf16 = mybir.dt.bfloat16
    AF = mybir.ActivationFunctionType
    ALU = mybir.AluOpType

    nc = bacc.Bacc(get_trn_type() or "TRN2", target_bir_lowering=False, debug=True)

    hf = nc.declare_dram_parameter("hf", [GPC * NPER, D], f32, isOutput=False)
    hs = nc.declare_dram_parameter("hs", [GPC * NPER, D], f32, isOutput=False)
    hff = nc.declare_dram_parameter("hff", [GPC, D], f32, isOutput=False)
    hsf = nc.declare_dram_parameter("hsf", [B, D], f32, isOutput=False)
    out_node = nc.declare_dram_parameter("out_node", [NPER, GPC], f32, isOutput=True)
    out_graph = nc.declare_dram_parameter("out_graph", [GPC, 1], f32, isOutput=True)

    eye_dram = nc.inline_tensor(np.eye(128, dtype=ml_dtypes.bfloat16), "eye_bf")

    with tile.TileContext(nc) as tc:
        with (
            tc.tile_pool(name="consts", bufs=1) as consts,
            tc.tile_pool(name="cols", bufs=1) as colsp,
            tc.tile_pool(name="loads", bufs=int(os.environ.get("K_LOADS", "4"))) as loads,
            tc.tile_pool(name="work", bufs=int(os.environ.get("K_WORK", "3"))) as work,
            tc.tile_pool(name="scr", bufs=int(os.environ.get("K_SCR", "2"))) as scr,
        ):
            ident = consts.tile([128, 128], bf16)
            nc.sync.dma_start(out=ident, in_=eye_dram[:, :])
            ones_c = consts.tile([128, 1], bf16)
            nc.vector.memset(ones_c, 1.0)
            lntau_c = consts.tile([128, 1], f32)
            nc.vector.memset(lntau_c, LN_INV_TAU)

            # per-graph column stats [128, GPC] f32
            ssq_f = colsp.tile([128, GPC], f32)
            ssq_s = colsp.tile([128, GPC], f32)
            invs_c = colsp.tile([128, GPC], f32)
            invf2_c = colsp.tile([128, GPC], f32)
            exppos_c = colsp.tile([128, GPC], f32)
            ln_scr = colsp.tile([128, GPC], f32)
            lnrow_c = colsp.tile([128, GPC], f32)
            pos_c = colsp.tile([128, GPC], f32)
            l_cols = colsp.tile([128, GPC], f32)

            # ---------------- graph-level loss (own PSUM scope) ----------------
            with (
                tc.tile_pool(name="fin", bufs=1) as fin,
                tc.tile_pool(name="fpsum", bufs=1, space="PSUM") as fpsum,
            ):
                hff_bf = fin.tile([GPC, D], bf16)
                nc.gpsimd.dma_start(out=hff_bf, in_=hff[:, :])
                hsf_bf = fin.tile([128, 4, D], bf16)
                nc.gpsimd.dma_start(
                    out=hsf_bf, in_=hsf[:, :].rearrange("(r p) d -> p r d", p=128)
                )

                ssq_ff = fin.tile([GPC, 1], f32)
                sqf_scr = fin.tile([GPC, D], bf16)
                nc.vector.scalar_tensor_tensor(
                    sqf_scr, hff_bf, 1.0, hff_bf,
                    op0=ALU.mult, op1=ALU.mult, accum_out=ssq_ff,
                )

                ssq_sf = fin.tile([128, 4], f32)
                sqs_scr = fin.tile([128, D], bf16)
                for r in range(4):
                    nc.vector.scalar_tensor_tensor(
                        sqs_scr, hsf_bf[:, r, :], 1.0, hsf_bf[:, r, :],
                        op0=ALU.mult, op1=ALU.mult,
                        accum_out=ssq_sf[:, r : r + 1],
                    )

                lnf_scr = fin.tile([128, 4], f32)
                invs_f = fin.tile([128, 4], f32)
                nc.scalar.activation(lnf_scr, ssq_sf, AF.Ln)
                nc.scalar.activation(invs_f, lnf_scr, AF.Exp, scale=-0.5)

                lnf2_scr = fin.tile([GPC, 1], f32)
                invf2_f = fin.tile([GPC, 1], f32)
                nc.scalar.activation(lnf2_scr, ssq_ff, AF.Ln)
                nc.scalar.activation(
                    invf2_f, lnf2_scr, AF.Exp, scale=-0.5, bias=lntau_c[:GPC]
                )

                hffN = fin.tile([GPC, D], bf16)
                nc.vector.tensor_scalar_mul(hffN, hff_bf, invf2_f)

                hffT = fin.tile([128, 2, GPC], bf16)
                nc.sync.dma_start(out=hffT, in_=hffN, transpose=True)
                hsfT = fin.tile([128, 4, 2, 128], bf16)
                nc.sync.dma_start(out=hsfT, in_=hsf_bf, transpose=True)

                # s_psf[j, r, i] = hs_j . hfN2_i  (raw hs: 1/|hs_j| via exp scale)
                s_psf = fpsum.tile([128, 4, GPC], f32)
                for r in range(4):
                    for c in range(2):
                        nc.tensor.matmul(
                            s_psf[:, r, :], hsfT[:, r, c, :], hffT[:, c, :],
                            start=(c == 0), stop=(c == 1),
                        )

                expf = fin.tile([128, 4, GPC], bf16)
                for r in range(4):
                    nc.scalar.activation(
                        expf[:, r, :], s_psf[:, r, :], AF.Exp,
                        scale=invs_f[:, r : r + 1],
                    )

                rowsumf_ps = fpsum.tile([GPC, 1], f32)
                for r in range(4):
                    nc.tensor.matmul(
                        rowsumf_ps, expf[:, r, :], ones_c,
                        start=(r == 0), stop=(r == 3),
                    )

                exppos_f = fin.tile([128, 1], f32)
                djunk_f = fin.tile([128, GPC], bf16)
                nc.vector.scalar_tensor_tensor(
                    djunk_f, expf[:, 0, :], 1.0, ident[:, :GPC],
                    op0=ALU.mult, op1=ALU.mult, accum_out=exppos_f,
                )

                lnrow_f = fin.tile([GPC, 1], f32)
                nc.scalar.activation(lnrow_f, rowsumf_ps, AF.Ln)
                pos_f = fin.tile([GPC, 1], f32)
                nc.scalar.activation(pos_f, exppos_f[:GPC, :], AF.Ln)
                lg = fin.tile([GPC, 1], f32)
                nc.vector.tensor_tensor(lg, lnrow_f, pos_f, op=ALU.subtract)
                nc.sync.dma_start(out=out_graph[:, :], in_=lg)

            # ---------------- node-level loss ----------------
            hf_r = hf[:, :].rearrange("(g p) d -> p g d", p=128)
            hs_r = hs[:, :].rearrange("(g p) d -> p g d", p=128)
            with (
                tc.tile_pool(name="spsum", bufs=int(os.environ.get("K_SPSUM", "2")), space="PSUM") as spsum,
                tc.tile_pool(name="rpsum", bufs=1, space="PSUM") as rpsum,
            ):
                rowsum_ps = rpsum.tile([128, GPC], f32)
                for b in range(NBLK):
                    bs = slice(b * BLK, (b + 1) * BLK)
                    hf_bf = loads.tile([128, BLK, D], bf16, tag="hf_bf")
                    nc.gpsimd.dma_start(out=hf_bf, in_=hf_r[:, bs, :])
                    hs_bf = loads.tile([128, BLK, D], bf16, tag="hs_bf")
                    nc.gpsimd.dma_start(out=hs_bf, in_=hs_r[:, bs, :])

                    for g in range(BLK):
                        gg = b * BLK + g
                        sqf_t = scr.tile([128, D], bf16, tag="sqf")
                        nc.vector.scalar_tensor_tensor(
                            sqf_t, hf_bf[:, g, :], 1.0, hf_bf[:, g, :],
                            op0=ALU.mult, op1=ALU.mult,
                            accum_out=ssq_f[:, gg : gg + 1],
                        )
                        sqs_t = scr.tile([128, D], bf16, tag="sqs")
                        nc.vector.scalar_tensor_tensor(
                            sqs_t, hs_bf[:, g, :], 1.0, hs_bf[:, g, :],
                            op0=ALU.mult, op1=ALU.mult,
                            accum_out=ssq_s[:, gg : gg + 1],
                        )

                    nc.scalar.activation(ln_scr[:, bs], ssq_s[:, bs], AF.Ln)
                    nc.scalar.activation(invs_c[:, bs], ln_scr[:, bs], AF.Exp, scale=-0.5)
                    nc.scalar.activation(ln_scr[:, bs], ssq_f[:, bs], AF.Ln)
                    nc.scalar.activation(
                        invf2_c[:, bs], ln_scr[:, bs], AF.Exp,
                        scale=-0.5, bias=lntau_c,
                    )

                    hfN = work.tile([128, BLK, D], bf16, tag="hfN")
                    for g in range(BLK):
                        gg = b * BLK + g
                        nc.vector.tensor_scalar_mul(
                            hfN[:, g, :], hf_bf[:, g, :], invf2_c[:, gg : gg + 1]
                        )

                    tT_f = work.tile([128, BLK, 2, 128], bf16, tag="tT_f")
                    tT_s = work.tile([128, BLK, 2, 128], bf16, tag="tT_s")
                    nc.sync.dma_start(out=tT_f, in_=hfN, transpose=True)
                    nc.sync.dma_start(out=tT_s, in_=hs_bf, transpose=True)
                    for q in range(BLK // QG):
                        s_ps = spsum.tile([128, QG, 128], f32, tag="s_ps")
                        exp_sb = work.tile([128, QG, 128], bf16, tag="exp_sb")
                        for j in range(QG):
                            g = q * QG + j
                            gg = b * BLK + g
                            for c in range(2):
                                nc.tensor.matmul(
                                    s_ps[:, j, :],
                                    tT_s[:, g, c, :],
                                    tT_f[:, g, c, :],
                                    start=(c == 0), stop=(c == 1),
                                )
                            nc.scalar.activation(
                                exp_sb[:, j, :], s_ps[:, j, :], AF.Exp,
                                scale=invs_c[:, gg : gg + 1],
                            )
                            djunk = scr.tile([128, 128], bf16, tag="djunk")
                            nc.vector.scalar_tensor_tensor(
                                djunk, exp_sb[:, j, :], 1.0, ident,
                                op0=ALU.mult, op1=ALU.mult,
                                accum_out=exppos_c[:, gg : gg + 1],
                            )
                            nc.tensor.matmul(
                                rowsum_ps[:, gg : gg + 1], exp_sb[:, j, :],
                                ones_c, start=True, stop=True,
                            )

                nc.scalar.activation(lnrow_c, rowsum_ps, AF.Ln)
                nc.scalar.activation(pos_c, exppos_c, AF.Ln)
                nc.vector.tensor_tensor(l_cols, lnrow_c, pos_c, op=ALU.subtract)
                nc.sync.dma_start(out=out_node[:, :], in_=l_cols)

    nc.compile()
    return nc


def _get_nc():
    if "nc" not in _CACHE:
        _CACHE["nc"] = _build()
    return _CACHE["nc"]


def _run(in_maps, **kwargs):
    from concourse.bass_utils import run_bass_kernel_spmd

    return run_bass_kernel_spmd(_get_nc(), in_maps, core_ids=list(range(NCORES)), **kwargs)


def make_in_maps(h_f_final, h_s_final, h_f, h_s):
    h_f = np.ascontiguousarray(np.asarray(h_f, dtype=np.float32))
    h_s = np.ascontiguousarray(np.asarray(h_s, dtype=np.float32))
    h_f_final = np.ascontiguousarray(np.asarray(h_f_final, dtype=np.float32))
    h_s_final = np.ascontiguousarray(np.asarray(h_s_final, dtype=np.float32))
    rows = GPC * NPER
    in_maps = []
    for c in range(NCORES):
        in_maps.append(
            {
                "hf": h_f[c * rows : (c + 1) * rows],
                "hs": h_s[c * rows : (c + 1) * rows],
                "hff": h_f_final[c * GPC : (c + 1) * GPC],
                "hsf": np.ascontiguousarray(np.roll(h_s_final, -GPC * c, axis=0)),
            }
        )
    return in_maps


def finish(results):
    l_node = np.concatenate(
        [r["out_node"].astype(np.float64).mean(axis=0) for r in results]
    )
    l_graph = np.concatenate([r["out_graph"][:, 0].astype(np.float64) for r in results])
    lam1 = l_node.std() + 1e-6
    lam2 = l_graph.std() + 1e-6
    return np.float32(lam1 * l_node.mean() + lam2 * l_graph.mean())


def kernel(h_f_final, h_s_final, h_f, h_s, batch=None, **_unused):
    res = _run(make_in_maps(h_f_final, h_s_final, h_f, h_s))
    return finish(res.results)


# revision 7
# speedup vs baseline: 1.0032x; 1.0032x over previous
"""CVTGAD loss kernel for 8 TRN2 NeuronCores.

Math (matches the jax reference):
  l_node[i,g] = ln(sum_j exp(simT[j,i])) - simT[i,i]   per graph (128x128 InfoNCE)
  l_graph     = InfoNCE over pooled graph embeddings (64 own rows x 512)
  out = (std(l_node)+1e-6) * mean(l_node) + (std(l_graph)+1e-6) * mean(l_graph)

Sharding: 64 graphs (8192 node rows) per core; h_s_final replicated (rolled per
core so each core's own graphs sit at columns 0:64, making the SPMD diag mask
core-independent). Device emits ln-rowsum and pos columns [128,64] plus
l_graph [64]; host does the tiny mean/std/weighted-sum epilogue.

Kernel strategy per core (tuned against the TimelineSim cost model):
  - SWDGE DMA-cast loads f32->bf16 (HBM reads billed on bf16 out bytes).
  - Row sumsq in ONE DVE pass per graph via scalar_tensor_tensor
    (out=(h*1)*h, accum_out=sum) -- InstTensorScalarPtr runs in 4x mode.
  - inv norms as exp(-0.5*ln(x)) on ACT, batched per block; 1/tau folded
    into hf's inv via the ln-bias trick.
  - hf normalized explicitly (DVE tensor_scalar 4x); hs stays raw and its
    1/|hs_j| rides the per-partition scale AP of the per-graph ACT Exp.
  - Gram computed TRANSPOSED (lhsT=hs^T chunk): s_ps[j,i] so that
  - rowsum over j = PE matmul with lhsT=exp_sb, rhs=ones -> psum column
    [128,1] per graph (ap_size=1: nearly free on the PE).
  - diagonal (pos) in ONE DVE 4x pass: scalar_tensor_tensor with in1=identity,
    accum_out = exp(pos).
  - l columns = ln(rowsum_psum) - ln(exppos) built with 2 batched ACT lns +
    1 DVE subtract; single [128,64] f32 store.
  - All activations pinned to the natural_log_exp_and_others table set
    (single ACT_TABLE_LOAD; the default picker thrashes ~2.7us reloads).
"""

import numpy as np

B = 512
NPER = 128
D = 256
NCORES = 8
GPC = B // NCORES      # 64 graphs per core
BLK = 8                # graphs per DMA block
NBLK = GPC // BLK
QG = 4                 # graphs per PSUM group (gram/exp granularity)
TAU = 0.5
LN_INV_TAU = float(np.log(1.0 / TAU))

_CACHE = {}


def _build():
    import os
    import ml_dtypes
    import concourse.bacc as bacc
    import concourse.tile as tile
    import concourse.mybir as mybir
    import concourse.hw_specs as hw_specs
    from concourse._compat import get_trn_type

    # Pin every activation to the one table set that has Exp+Ln+Square+Copy,
    # so the compiler emits a single ACT_TABLE_LOAD instead of thrashing
    # (each reload costs ~2.7us and the default picker alternates sets).
    if not getattr(hw_specs, "_nle_patched", False):
        _orig_tables = hw_specs.get_activation_tables

        def _only_nle(arch):
            t = _orig_tables(arch)
            keep = "natural_log_exp_and_others"
            return {k: (v if k == keep else set()) for k, v in t.items()}

        hw_specs.get_activation_tables = _only_nle
        bacc.get_activation_tables = _only_nle
        hw_specs._nle_patched = True

    f32 = mybir.dt.float32
    bf16 = mybir.dt.bfloat16
    AF = mybir.ActivationFunctionType
    ALU = mybir.AluOpType

    nc = bacc.Bacc(get_trn_type() or "TRN2", target_bir_lowering=False, debug=True)

    hf = nc.declare_dram_parameter("hf", [GPC * NPER, D], f32, isOutput=False)
    hs = nc.declare_dram_parameter("hs", [GPC * NPER, D], f32, isOutput=False)
    hff = nc.declare_dram_parameter("hff", [GPC, D], f32, isOutput=False)
    hsf = nc.declare_dram_parameter("hsf", [B, D], f32, isOutput=False)
    out_node = nc.declare_dram_parameter("out_node", [NPER, GPC], f32, isOutput=True)
    out_graph = nc.declare_dram_parameter("out_graph", [GPC, 1], f32, isOutput=True)

    eye_dram = nc.inline_tensor(np.eye(128, dtype=ml_dtypes.bfloat16), "eye_bf")

    with tile.TileContext(nc) as tc:
        with (
            tc.tile_pool(name="consts", bufs=1) as consts,
            tc.tile_pool(name="cols", bufs=1) as colsp,
            tc.tile_pool(name="loads", bufs=int(os.environ.get("K_LOADS", "4"))) as loads,
            tc.tile_pool(name="work", bufs=int(os.environ.get("K_WORK", "3"))) as work,
            tc.tile_pool(name="scr", bufs=int(os.environ.get("K_SCR", "2"))) as scr,
        ):
            ident = consts.tile([128, 128], bf16)
            nc.sync.dma_start(out=ident, in_=eye_dram[:, :])
            ones_c = consts.tile([128, 1], bf16)
            nc.vector.memset(ones_c, 1.0)
            lntau_c = consts.tile([128, 1], f32)
            nc.vector.memset(lntau_c, LN_INV_TAU)

            # per-graph column stats [128, GPC] f32
            ssq_f = colsp.tile([128, GPC], f32)
            ssq_s = colsp.tile([128, GPC], f32)
            invs_c = colsp.tile([128, GPC], f32)
            invf2_c = colsp.tile([128, GPC], f32)
            exppos_c = colsp.tile([128, GPC], f32)
            ln_scr = colsp.tile([128, GPC], f32)
            lnrow_c = colsp.tile([128, GPC], f32)
            pos_c = colsp.tile([128, GPC], f32)
            l_cols = colsp.tile([128, GPC], f32)

            # ---------------- graph-level loss (own PSUM scope) ----------------
            with (
                tc.tile_pool(name="fin", bufs=1) as fin,
                tc.tile_pool(name="fpsum", bufs=1, space="PSUM") as fpsum,
            ):
                hff_bf = fin.tile([GPC, D], bf16)
                nc.gpsimd.dma_start(out=hff_bf, in_=hff[:, :])
                hsf_bf = fin.tile([128, 4, D], bf16)
                nc.gpsimd.dma_start(
                    out=hsf_bf, in_=hsf[:, :].rearrange("(r p) d -> p r d", p=128)
                )

                ssq_ff = fin.tile([GPC, 1], f32)
                sqf_raw = fin.tile([GPC, D], bf16)
                nc.vector.tensor_tensor(sqf_raw, hff_bf, hff_bf, op=ALU.mult)
                sqf_scr = fin.tile([GPC, D], bf16)
                nc.vector.tensor_scalar(
                    sqf_scr, sqf_raw, 1.0, 0.0,
                    op0=ALU.mult, op1=ALU.add, accum_out=ssq_ff,
                )

                ssq_sf = fin.tile([128, 4], f32)
                sqs_raw = fin.tile([128, 4, D], bf16)
                nc.vector.tensor_tensor(sqs_raw, hsf_bf, hsf_bf, op=ALU.mult)
                sqs_scr = fin.tile([128, D], bf16)
                for r in range(4):
                    nc.vector.tensor_scalar(
                        sqs_scr, sqs_raw[:, r, :], 1.0, 0.0,
                        op0=ALU.mult, op1=ALU.add,
                        accum_out=ssq_sf[:, r : r + 1],
                    )

                lnf_scr = fin.tile([128, 4], f32)
                invs_f = fin.tile([128, 4], f32)
                nc.scalar.activation(lnf_scr, ssq_sf, AF.Ln)
                nc.scalar.activation(invs_f, lnf_scr, AF.Exp, scale=-0.5)

                lnf2_scr = fin.tile([GPC, 1], f32)
                invf2_f = fin.tile([GPC, 1], f32)
                nc.scalar.activation(lnf2_scr, ssq_ff, AF.Ln)
                nc.scalar.activation(
                    invf2_f, lnf2_scr, AF.Exp, scale=-0.5, bias=lntau_c[:GPC]
                )

                hffN = fin.tile([GPC, D], bf16)
                nc.vector.tensor_scalar_mul(hffN, hff_bf, invf2_f)

                hffT = fin.tile([128, 2, GPC], bf16)
                nc.sync.dma_start(out=hffT, in_=hffN, transpose=True)
                hsfT = fin.tile([128, 4, 2, 128], bf16)
                nc.sync.dma_start(out=hsfT, in_=hsf_bf, transpose=True)

                # s_psf[j, r, i] = hs_j . hfN2_i  (raw hs: 1/|hs_j| via exp scale)
                s_psf = fpsum.tile([128, 4, GPC], f32)
                for r in range(4):
                    for c in range(2):
                        nc.tensor.matmul(
                            s_psf[:, r, :], hsfT[:, r, c, :], hffT[:, c, :],
                            start=(c == 0), stop=(c == 1),
                        )

                expf = fin.tile([128, 4, GPC], bf16)
                for r in range(4):
                    nc.scalar.activation(
                        expf[:, r, :], s_psf[:, r, :], AF.Exp,
                        scale=invs_f[:, r : r + 1],
                    )

                rowsumf_ps = fpsum.tile([GPC, 1], f32)
                for r in range(4):
                    nc.tensor.matmul(
                        rowsumf_ps, expf[:, r, :], ones_c,
                        start=(r == 0), stop=(r == 3),
                    )

                exppos_f = fin.tile([128, 1], f32)
                dmask_f = fin.tile([128, GPC], bf16)
                nc.vector.tensor_tensor(
                    dmask_f, expf[:, 0, :], ident[:, :GPC], op=ALU.mult
                )
                djunk_f = fin.tile([128, GPC], bf16)
                nc.vector.tensor_scalar(
                    djunk_f, dmask_f, 1.0, 0.0,
                    op0=ALU.mult, op1=ALU.add, accum_out=exppos_f,
                )

                lnrow_f = fin.tile([GPC, 1], f32)
                nc.scalar.activation(lnrow_f, rowsumf_ps, AF.Ln)
                pos_f = fin.tile([GPC, 1], f32)
                nc.scalar.activation(pos_f, exppos_f[:GPC, :], AF.Ln)
                lg = fin.tile([GPC, 1], f32)
                nc.vector.tensor_tensor(lg, lnrow_f, pos_f, op=ALU.subtract)
                nc.sync.dma_start(out=out_graph[:, :], in_=lg)

            # ---------------- node-level loss ----------------
            hf_r = hf[:, :].rearrange("(g p) d -> p g d", p=128)
            hs_r = hs[:, :].rearrange("(g p) d -> p g d", p=128)
            with (
                tc.tile_pool(name="spsum", bufs=int(os.environ.get("K_SPSUM", "2")), space="PSUM") as spsum,
                tc.tile_pool(name="rpsum", bufs=1, space="PSUM") as rpsum,
            ):
                rowsum_ps = rpsum.tile([128, GPC], f32)

                def drain(pending):
                    # rowsum + diag for a PSUM group, deferred one group so the
                    # PE/DVE streams never stall on the ACT exps of their own
                    # group (classic software pipelining).
                    gg0, exp_prev = pending
                    for j in range(QG):
                        gg = gg0 + j
                        nc.tensor.matmul(
                            rowsum_ps[:, gg : gg + 1], exp_prev[:, j, :],
                            ones_c, start=True, stop=True,
                        )
                        dmask = scr.tile([128, 128], bf16, tag="dmask")
                        nc.vector.tensor_tensor(
                            dmask, exp_prev[:, j, :], ident, op=ALU.mult
                        )
                        djunk = scr.tile([128, 128], bf16, tag="djunk")
                        nc.vector.tensor_scalar(
                            djunk, dmask, 1.0, 0.0,
                            op0=ALU.mult, op1=ALU.add,
                            accum_out=exppos_c[:, gg : gg + 1],
                        )

                pending = None
                for b in range(NBLK):
                    bs = slice(b * BLK, (b + 1) * BLK)
                    hf_bf = loads.tile([128, BLK, D], bf16, tag="hf_bf")
                    nc.gpsimd.dma_start(out=hf_bf, in_=hf_r[:, bs, :])
                    hs_bf = loads.tile([128, BLK, D], bf16, tag="hs_bf")
                    nc.gpsimd.dma_start(out=hs_bf, in_=hs_r[:, bs, :])

                    sq_f = scr.tile([128, BLK, D], bf16, tag="sqf")
                    nc.vector.tensor_tensor(sq_f, hf_bf, hf_bf, op=ALU.mult)
                    sq_s = scr.tile([128, BLK, D], bf16, tag="sqs")
                    nc.vector.tensor_tensor(sq_s, hs_bf, hs_bf, op=ALU.mult)
                    for g in range(BLK):
                        gg = b * BLK + g
                        acc_t = scr.tile([128, D], bf16, tag="acc")
                        nc.vector.tensor_scalar(
                            acc_t, sq_f[:, g, :], 1.0, 0.0,
                            op0=ALU.mult, op1=ALU.add,
                            accum_out=ssq_f[:, gg : gg + 1],
                        )
                        acc_t2 = scr.tile([128, D], bf16, tag="acc2")
                        nc.vector.tensor_scalar(
                            acc_t2, sq_s[:, g, :], 1.0, 0.0,
                            op0=ALU.mult, op1=ALU.add,
                            accum_out=ssq_s[:, gg : gg + 1],
                        )

                    nc.scalar.activation(ln_scr[:, bs], ssq_s[:, bs], AF.Ln)
                    nc.scalar.activation(invs_c[:, bs], ln_scr[:, bs], AF.Exp, scale=-0.5)
                    nc.scalar.activation(ln_scr[:, bs], ssq_f[:, bs], AF.Ln)
                    nc.scalar.activation(
                        invf2_c[:, bs], ln_scr[:, bs], AF.Exp,
                        scale=-0.5, bias=lntau_c,
                    )

                    hfN = work.tile([128, BLK, D], bf16, tag="hfN")
                    for g in range(BLK):
                        gg = b * BLK + g
                        nc.vector.tensor_scalar_mul(
                            hfN[:, g, :], hf_bf[:, g, :], invf2_c[:, gg : gg + 1]
                        )

                    tT_f = work.tile([128, BLK, 2, 128], bf16, tag="tT_f")
                    tT_s = work.tile([128, BLK, 2, 128], bf16, tag="tT_s")
                    nc.sync.dma_start(out=tT_f, in_=hfN, transpose=True)
                    nc.sync.dma_start(out=tT_s, in_=hs_bf, transpose=True)
                    for q in range(BLK // QG):
                        s_ps = spsum.tile([128, QG, 128], f32, tag="s_ps")
                        exp_sb = work.tile([128, QG, 128], bf16, tag="exp_sb")
                        for j in range(QG):
                            g = q * QG + j
                            for c in range(2):
                                nc.tensor.matmul(
                                    s_ps[:, j, :],
                                    tT_s[:, g, c, :],
                                    tT_f[:, g, c, :],
                                    start=(c == 0), stop=(c == 1),
                                )
                        for j in range(QG):
                            gg = b * BLK + q * QG + j
                            nc.scalar.activation(
                                exp_sb[:, j, :], s_ps[:, j, :], AF.Exp,
                                scale=invs_c[:, gg : gg + 1],
                            )
                        if pending is not None:
                            drain(pending)
                        pending = (b * BLK + q * QG, exp_sb)
                if pending is not None:
                    drain(pending)

                nc.scalar.activation(lnrow_c, rowsum_ps, AF.Ln)
                nc.scalar.activation(pos_c, exppos_c, AF.Ln)
                nc.vector.tensor_tensor(l_cols, lnrow_c, pos_c, op=ALU.subtract)
                nc.sync.dma_start(out=out_node[:, :], in_=l_cols)

    nc.compile()
    return nc


def _get_nc():
    if "nc" not in _CACHE:
        _CACHE["nc"] = _build()
    return _CACHE["nc"]


def _run(in_maps, **kwargs):
    from concourse.bass_utils import run_bass_kernel_spmd

    return run_bass_kernel_spmd(_get_nc(), in_maps, core_ids=list(range(NCORES)), **kwargs)


def make_in_maps(h_f_final, h_s_final, h_f, h_s):
    h_f = np.ascontiguousarray(np.asarray(h_f, dtype=np.float32))
    h_s = np.ascontiguousarray(np.asarray(h_s, dtype=np.float32))
    h_f_final = np.ascontiguousarray(np.asarray(h_f_final, dtype=np.float32))
    h_s_final = np.ascontiguousarray(np.asarray(h_s_final, dtype=np.float32))
    rows = GPC * NPER
    in_maps = []
    for c in range(NCORES):
        in_maps.append(
            {
                "hf": h_f[c * rows : (c + 1) * rows],
                "hs": h_s[c * rows : (c + 1) * rows],
                "hff": h_f_final[c * GPC : (c + 1) * GPC],
                "hsf": np.ascontiguousarray(np.roll(h_s_final, -GPC * c, axis=0)),
            }
        )
    return in_maps


def finish(results):
    l_node = np.concatenate(
        [r["out_node"].astype(np.float64).mean(axis=0) for r in results]
    )
    l_graph = np.concatenate([r["out_graph"][:, 0].astype(np.float64) for r in results])
    lam1 = l_node.std() + 1e-6
    lam2 = l_graph.std() + 1e-6
    return np.float32(lam1 * l_node.mean() + lam2 * l_graph.mean())


def kernel(h_f_final, h_s_final, h_f, h_s, batch=None, **_unused):
    res = _run(make_in_maps(h_f_final, h_s_final, h_f, h_s))
    return finish(res.results)
